# revision 17
# baseline (speedup 1.0000x reference)
"""Trainium2 Bass kernel for nn_ContrastiveLoss (NT-Xent-style loss with
tag/document masking).

Strategy (8 NeuronCores, SPMD), V9:
  - Symmetric-half decomposition: each 128-row tile computes rolled cols
    [i*128, i*128+4096) = its self block + 31 forward blocks.  Row-sums
    ship directly; column-sums of the computed half (SBUF f16 accumulator)
    supply the missing halves of other rows' sums (host adds them and does
    the 128-partition reduce).
  - The antipodal band (block distance exactly 32; 1/32nd of the matrix)
    is computed exactly on the host along with the partner-numerator.
  - PSUM = two [128,2048] spans (4 banks each, one pool, 2 allocs/tile):
    per span 4 sim-DR matmuls + 4 single-plane mask matmuls, then ONE
    2048-wide exp with accum_out row-sums (~2.05us steady cadence).
  - Masking via a SINGLE penalty K-plane: rows 0-63 one-hot tag%64,
    rows 64-127 one-hot 64+doc%64 (two ones per column);
    PSUM = 16*sim - 240*[tagslot_eq] - 240*[docslot_eq].  The mask
    matmul is plain fp8 K=128 (FWL fast weight load, no DoubleRow).
    Over-masking ~2.1% of pairs -> ~2.3e-3 relative loss error.
  - Inputs: THREE planes (q1 x2 + mask x1) packed as two whole-tile
    DRAM tensors qqA (cols [0,2944)) / qqB (cols [2048,4992)) so each is
    ONE fully-contiguous DMA (11KB/partition descriptors, full HBM BW),
    serialized on the Sync ring; p2 on the Scalar ring.
  - colacc: one tensor_tensor f16 add per SPAN; parity-split buffers;
    colO ships in 3 pieces (after tile 5, after tile 7's A-half, end),
    colE after tile 6; row-sum reduce issued before the final TT.
"""

import sys

for _p in ("/opt/trn_rl_repo", "/root/.axon_site/_ro/trn_rl_repo"):
    if _p not in sys.path:
        sys.path.insert(0, _p)

from contextlib import ExitStack

import ml_dtypes
import numpy as np

from concourse import bacc, mybir, tile
from concourse.bass_utils import run_bass_kernel_spmd

F32 = mybir.dt.float32
F16 = mybir.dt.float16
F8 = mybir.dt.float8e4
FP8NP = ml_dtypes.float8_e4m3fn

P = 128          # SBUF partitions
B = 4096         # batch
D = 256          # embedding dim
N = 2 * B        # 8192 rows/cols of the similarity matrix
CORES = 8
ROWS_PER_CORE = N // CORES      # 1024
NI = ROWS_PER_CORE // P         # 8 row tiles per core
CH = 512                        # column chunk (one PSUM bank of fp32)
NC = 8                          # column chunks per row tile
W = NC * CH                     # 4096 columns in a row tile's window
NLOAD = (NI - 1) * P + W        # 4992 cols actually read per core
SPAN = 4 * CH                   # 2048-wide PSUM span (4 banks)
QW = (NI - 1) * P + SPAN        # 2944: width of each half-input tensor
QB0 = SPAN                      # qqB covers global cols [2048, 4992)
RSCALE = 4.0                    # rep pre-scale; sim comes out as 16*sim
TS = 0.125                      # exp scale: exp(0.125 * PSUM)
PEN = -240.0                    # mask penalty per onehot hit (0.125*240=30)
TEMPERATURE = 0.5


def _build_program(debug=False):
    nc = bacc.Bacc("TRN2" if debug else None, target_bir_lowering=False,
                   debug=debug)

    # four whole-tile input pieces (each fully contiguous; consumers wait
    # for complete DMAs, so smaller pieces unblock the pipeline earlier):
    # A0 = global cols [0,1920), A1 = [1024,2944), B0 = [2048,3968),
    # B1 = [3072,4992)
    qp_d = [nc.declare_dram_parameter(f"qp{s}", [P, 3, 1920], F8,
                                      isOutput=False) for s in range(4)]
    p2_d = nc.declare_dram_parameter("p2", [P, ROWS_PER_CORE], F8,
                                     isOutput=False)
    out_d = nc.declare_dram_parameter("out", [P, NI], F32, isOutput=True)
    colE_d = nc.declare_dram_parameter("colE", [P, NLOAD], F16, isOutput=True)
    colO_d = nc.declare_dram_parameter("colO", [P, NLOAD], F16, isOutput=True)

    Exp = mybir.ActivationFunctionType.Exp
    add = mybir.AluOpType.add
    DR = mybir.MatmulPerfMode.DoubleRow

    with tile.TileContext(nc) as tc, ExitStack() as ctx:
        persist = ctx.enter_context(tc.tile_pool(name="persist", bufs=1))
        qp = [persist.tile([P, 3, 1920], F8, tag=f"qp{s}", name=f"qp{s}")
              for s in range(4)]
        p2 = persist.tile([P, ROWS_PER_CORE], F8, tag="p2")
        v_sb = persist.tile([P, NI], F32, tag="v_sb")
        colE = persist.tile([P, NLOAD], F16, tag="colE")
        colO = persist.tile([P, NLOAD], F16, tag="colO")

        # Whole-tile input DMAs (contiguous both sides -> full HBM BW),
        # arrival-ordered on the Sync ring so tile 0 starts ASAP.
        nc.sync.dma_start(qp[0][:], qp_d[0][:])
        nc.scalar.dma_start(p2[:], p2_d[:])
        for s in range(1, 4):
            nc.sync.dma_start(qp[s][:], qp_d[s][:])
        nc.vector.memset(colE[:], 0.0)
        nc.vector.memset(colO[:], 0.0)

        with (
            tc.tile_pool(name="work", bufs=2) as work,
            tc.tile_pool(name="acc", bufs=2) as accp,
            tc.tile_pool(name="ps", bufs=2, space="PSUM") as ps,
        ):
            for i in range(NI):
                ms = slice(i * P, (i + 1) * P)
                c0 = i * P
                Et = work.tile([P, W], F16, tag="Et")
                sall = accp.tile([P, 2], F32, tag="sall")
                colX = colE if i % 2 == 0 else colO

                for half in range(2):
                    Sp = ps.tile([P, SPAN], F32, tag="S",
                                 name=f"S{i}_{half}")
                    ks = range(4 * half, 4 * half + 4)
                    # piece s covers global cols [1024*s, 1024*s + 1920)
                    def _rhs(k):
                        s = k // 2
                        return qp[s], c0 + k * CH - 1024 * s
                    for k in ks:
                        qqX, j0 = _rhs(k)
                        nc.tensor.matmul(
                            Sp[:, (k % 4) * CH:(k % 4 + 1) * CH],
                            qp[0][:, 0:2, ms], qqX[:, 0:2, j0:j0 + CH],
                            start=True, stop=False, perf_mode=DR,
                        )
                    for k in ks:
                        qqX, j0 = _rhs(k)
                        nc.tensor.matmul(
                            Sp[:, (k % 4) * CH:(k % 4 + 1) * CH],
                            p2[:, ms], qqX[:, 2:3, j0:j0 + CH],
                            start=False, stop=True,
                        )
                    nc.scalar.activation(
                        Et[:, half * SPAN:(half + 1) * SPAN], Sp[:], Exp,
                        scale=TS, accum_out=sall[:, half:half + 1])
                    if half == 0:
                        # A-half column accumulation (self block excluded)
                        nc.vector.tensor_tensor(
                            colX[:, c0 + P:c0 + SPAN], Et[:, P:SPAN],
                            colX[:, c0 + P:c0 + SPAN], add)

                nc.vector.tensor_reduce(
                    v_sb[:, i:i + 1], sall[:], mybir.AxisListType.X, add)
                nc.vector.tensor_tensor(
                    colX[:, c0 + SPAN:c0 + W], Et[:, SPAN:W],
                    colX[:, c0 + SPAN:c0 + W], add)

                if i == 5:
                    nc.sync.dma_start(colO_d[:, 0:1024], colO[:, 0:1024])
                if i == NI - 2:
                    nc.sync.dma_start(colE_d[:], colE[:])

            nc.scalar.dma_start(out_d[:], v_sb[:])
            nc.sync.dma_start(colO_d[:, 1024:NLOAD], colO[:, 1024:NLOAD])

    nc.compile()
    return nc


_NC_CACHE = []


def _get_nc():
    if not _NC_CACHE:
        _NC_CACHE.append(_build_program())
    return _NC_CACHE[0]


def _prepare_inputs(emb_i, emb_j, tags, document_ids):
    emb_i = np.asarray(emb_i, dtype=np.float32)
    emb_j = np.asarray(emb_j, dtype=np.float32)
    z_i = emb_i / np.linalg.norm(emb_i, axis=1, keepdims=True)
    z_j = emb_j / np.linalg.norm(emb_j, axis=1, keepdims=True)
    reps = np.concatenate([z_i, z_j], axis=0)                    # [N, 256]
    repsT = reps.T * RSCALE                                      # [256, N]
    tags2 = np.concatenate([tags, tags]).astype(np.int64)        # [8192]
    docs2 = np.concatenate([document_ids, document_ids]).astype(np.int64)

    # DoubleRow plane layout: element (p, pl, n) is contraction row pl*128+p
    q1_full = repsT.reshape(2, P, N).transpose(1, 0, 2)          # [128,2,N]

    # single mask plane: rows 0-63 one-hot tag%64, rows 64-127 doc%64
    q2h = np.zeros((P, 1, N), dtype=np.float32)
    q2h[tags2 % 64, 0, np.arange(N)] = 1.0
    q2h[64 + docs2 % 64, 0, np.arange(N)] = 1.0

    qq_full = np.concatenate([q1_full, q2h], axis=1).astype(FP8NP)

    in_maps = []
    for c in range(CORES):
        r = c * ROWS_PER_CORE
        roll = np.r_[r:N, 0:r][:NLOAD]
        qq_c = qq_full[:, :, roll]
        im = {"p2": np.ascontiguousarray(
            q2h[:, 0, roll[:ROWS_PER_CORE]] * PEN).astype(FP8NP)}
        for s in range(4):
            im[f"qp{s}"] = np.ascontiguousarray(
                qq_c[:, :, 1024 * s:1024 * s + 1920])
        in_maps.append(im)

    # Host side: exact partner numerator + the antipodal band (block
    # distance exactly 32), which the device skips.
    z_pair_sim = np.einsum("ij,ij->i", z_i.astype(np.float64),
                           z_j.astype(np.float64))               # [B]
    zb = reps.reshape(N // P, P, D)                              # [64,128,256]
    sim_anti = np.einsum("bij,bkj->bik", zb[:N // P // 2],
                         zb[N // P // 2:]).astype(np.float64)    # [32,128,128]
    e_anti = np.exp(sim_anti / TEMPERATURE)
    tb = tags2.reshape(N // P, P)
    db = docs2.reshape(N // P, P)
    half = N // P // 2
    m = ((tb[:half, :, None] != tb[half:, None, :])
         & (db[:half, :, None] != db[half:, None, :])).astype(np.float64)
    me = m * e_anti
    anti_rowsum = np.concatenate(
        [me.sum(axis=2).reshape(-1), me.sum(axis=1).reshape(-1)])  # [N]
    return in_maps, (z_pair_sim, anti_rowsum)


def _assemble_loss(results, host_extra):
    z_pair_sim, anti_rowsum = host_extra
    rowsum = anti_rowsum.copy()
    w = np.arange(NLOAD)
    for c in range(CORES):
        r = c * ROWS_PER_CORE
        o = np.asarray(results[c]["out"]).astype(np.float64)     # [P, NI]
        rows = r + np.arange(ROWS_PER_CORE)
        rowsum[rows] += o.T.reshape(-1)
        colsum = (np.asarray(results[c]["colE"]).astype(np.float64).sum(0)
                  + np.asarray(results[c]["colO"]).astype(np.float64).sum(0))
        np.add.at(rowsum, (r + w) % N, colsum)
    denom = rowsum + 0.1
    # numerator: exact partner similarity, log(exp(sim/T)) = sim/T
    simfull = np.concatenate([z_pair_sim, z_pair_sim])
    v = np.log(denom) - simfull / TEMPERATURE
    return np.float32(v.sum() / N)


def kernel(emb_i, emb_j, tags, num_classes, document_ids):
    nc = _get_nc()
    in_maps, host_extra = _prepare_inputs(emb_i, emb_j, tags, document_ids)
    res = run_bass_kernel_spmd(nc, in_maps, list(range(CORES)))
    return _assemble_loss(res.results, host_extra)


# revision 18
# speedup vs baseline: 1.0191x; 1.0191x over previous
"""Trainium2 Bass kernel for nn_ContrastiveLoss (NT-Xent-style loss with
tag/document masking).

Strategy (8 NeuronCores, SPMD), V8 (final):
  - Symmetric-half decomposition: each 128-row tile computes rolled cols
    [i*128, i*128+4096) = its self block + 31 forward blocks.  Row-sums
    ship directly; column-sums of the computed half (SBUF f16 accumulator)
    supply the missing halves of other rows' sums (host adds them and does
    the 128-partition reduce).
  - The antipodal band (block distance exactly 32; 1/32nd of the matrix)
    is computed exactly on the host along with the partner-numerator.
  - PSUM = two [128,2048] spans (4 banks each, one pool, 2 allocs/tile):
    per span 4 sim-DR + 4 mask matmuls, then ONE 2048-wide exp with
    accum_out row-sums (ACT accumulator; ~2.05us steady cadence).
  - ALL masking fused into the matmul via penalty K-planes:
    PSUM = 16*sim - 240*[tag_eq] - 240*[doclo_eq],  doclo = doc mod 128.
  - Inputs stage-major: q1+q2 packed into per-stage CONTIGUOUS DRAM
    tensors (qs0/qs1/qs2) so each stage DMA moves contiguous
    per-partition runs; triggers alternate the Sync/Scalar HWDGE rings
    (each trigger ~640ns serialized on its issuing queue).
  - colacc: one tensor_tensor f16 add per SPAN (starts right after that
    span's exp); parity-split buffers; colE ships after tile 6,
    colO[0:1024] after tile 5, remainder at the end.
"""

import sys

for _p in ("/opt/trn_rl_repo", "/root/.axon_site/_ro/trn_rl_repo"):
    if _p not in sys.path:
        sys.path.insert(0, _p)

from contextlib import ExitStack

import ml_dtypes
import numpy as np

from concourse import bacc, mybir, tile
from concourse.bass_utils import run_bass_kernel_spmd

F32 = mybir.dt.float32
F16 = mybir.dt.float16
F8 = mybir.dt.float8e4
FP8NP = ml_dtypes.float8_e4m3fn

P = 128          # SBUF partitions
B = 4096         # batch
D = 256          # embedding dim
N = 2 * B        # 8192 rows/cols of the similarity matrix
CORES = 8
ROWS_PER_CORE = N // CORES      # 1024
NI = ROWS_PER_CORE // P         # 8 row tiles per core
CH = 512                        # column chunk (one PSUM bank of fp32)
NC = 8                          # column chunks per row tile
W = NC * CH                     # 4096 columns in a row tile's window
NLOAD = (NI - 1) * P + W        # 4992 cols actually read per core
SPAN = 4 * CH                   # 2048-wide PSUM span (4 banks)
STAGES = [(0, 2048), (2048, 4096), (4096, NLOAD)]
RSCALE = 4.0                    # rep pre-scale; sim comes out as 16*sim
TS = 0.125                      # exp scale: exp(0.125 * PSUM)
PEN = -240.0                    # mask penalty per onehot plane (0.125*240=30)
TEMPERATURE = 0.5


def _build_program(debug=False):
    nc = bacc.Bacc("TRN2" if debug else None, target_bir_lowering=False,
                   debug=debug)

    qs_d = [
        nc.declare_dram_parameter(f"qs{s}", [P, 4, b - a], F8, isOutput=False)
        for s, (a, b) in enumerate(STAGES)
    ]
    p2_d = nc.declare_dram_parameter("p2", [P, 2, ROWS_PER_CORE], F8,
                                     isOutput=False)
    out_d = nc.declare_dram_parameter("out", [P, NI], F32, isOutput=True)
    colE_d = nc.declare_dram_parameter("colE", [P, NLOAD], F16, isOutput=True)
    colO_d = nc.declare_dram_parameter("colO", [P, NLOAD], F16, isOutput=True)

    Exp = mybir.ActivationFunctionType.Exp
    add = mybir.AluOpType.add
    DR = mybir.MatmulPerfMode.DoubleRow

    with tile.TileContext(nc) as tc, ExitStack() as ctx:
        persist = ctx.enter_context(tc.tile_pool(name="persist", bufs=1))
        qq = persist.tile([P, 4, NLOAD], F8, tag="qq")
        p2 = persist.tile([P, 2, ROWS_PER_CORE], F8, tag="p2")
        v_sb = persist.tile([P, NI], F32, tag="v_sb")
        colE = persist.tile([P, NLOAD], F16, tag="colE")
        colO = persist.tile([P, NLOAD], F16, tag="colO")

        # Staged input DMA on alternating HWDGE rings; each stage's DRAM
        # side is fully contiguous per partition.
        nc.sync.dma_start(qq[:, :, STAGES[0][0]:STAGES[0][1]], qs_d[0][:])
        nc.scalar.dma_start(p2[:], p2_d[:])
        nc.scalar.dma_start(qq[:, :, STAGES[1][0]:STAGES[1][1]], qs_d[1][:])
        nc.sync.dma_start(qq[:, :, STAGES[2][0]:STAGES[2][1]], qs_d[2][:])
        nc.vector.memset(colE[:], 0.0)
        nc.vector.memset(colO[:], 0.0)

        with (
            tc.tile_pool(name="work", bufs=2) as work,
            tc.tile_pool(name="acc", bufs=2) as accp,
            tc.tile_pool(name="ps", bufs=2, space="PSUM") as ps,
        ):
            for i in range(NI):
                ms = slice(i * P, (i + 1) * P)
                c0 = i * P
                Et = work.tile([P, W], F16, tag="Et")
                sall = accp.tile([P, 2], F32, tag="sall")
                colX = colE if i % 2 == 0 else colO

                for half in range(2):
                    Sp = ps.tile([P, SPAN], F32, tag="S",
                                 name=f"S{i}_{half}")
                    ks = range(4 * half, 4 * half + 4)
                    for k in ks:
                        js = slice(c0 + k * CH, c0 + (k + 1) * CH)
                        nc.tensor.matmul(
                            Sp[:, (k % 4) * CH:(k % 4 + 1) * CH],
                            qq[:, 0:2, ms], qq[:, 0:2, js],
                            start=True, stop=False, perf_mode=DR,
                        )
                    for k in ks:
                        js = slice(c0 + k * CH, c0 + (k + 1) * CH)
                        nc.tensor.matmul(
                            Sp[:, (k % 4) * CH:(k % 4 + 1) * CH],
                            p2[:, :, ms], qq[:, 2:4, js],
                            start=False, stop=True, perf_mode=DR,
                        )
                    nc.scalar.activation(
                        Et[:, half * SPAN:(half + 1) * SPAN], Sp[:], Exp,
                        scale=TS, accum_out=sall[:, half:half + 1])
                    # column accumulation per span (self block excluded)
                    lo = P if half == 0 else SPAN
                    nc.vector.tensor_tensor(
                        colX[:, c0 + lo:c0 + (half + 1) * SPAN],
                        Et[:, lo:(half + 1) * SPAN],
                        colX[:, c0 + lo:c0 + (half + 1) * SPAN], add)

                nc.vector.tensor_reduce(
                    v_sb[:, i:i + 1], sall[:], mybir.AxisListType.X, add)

                if i == 5:
                    nc.sync.dma_start(colO_d[:, 0:1024], colO[:, 0:1024])
                if i == NI - 2:
                    nc.sync.dma_start(colE_d[:], colE[:])

            nc.sync.dma_start(colO_d[:, 1024:NLOAD], colO[:, 1024:NLOAD])
            nc.scalar.dma_start(out_d[:], v_sb[:])

    nc.compile()
    return nc


_NC_CACHE = []


def _get_nc():
    if not _NC_CACHE:
        _NC_CACHE.append(_build_program())
    return _NC_CACHE[0]


def _prepare_inputs(emb_i, emb_j, tags, document_ids):
    emb_i = np.asarray(emb_i, dtype=np.float32)
    emb_j = np.asarray(emb_j, dtype=np.float32)
    z_i = emb_i / np.linalg.norm(emb_i, axis=1, keepdims=True)
    z_j = emb_j / np.linalg.norm(emb_j, axis=1, keepdims=True)
    reps = np.concatenate([z_i, z_j], axis=0)                    # [N, 256]
    repsT = reps.T * RSCALE                                      # [256, N]
    tags2 = np.concatenate([tags, tags]).astype(np.int64)        # [8192]
    docs2 = np.concatenate([document_ids, document_ids]).astype(np.int64)
    doclo = (docs2 % P).astype(np.int64)

    # DoubleRow plane layout: element (p, pl, n) is contraction row pl*128+p
    q1_full = repsT.reshape(2, P, N).transpose(1, 0, 2)          # [128,2,N]

    q2f = np.zeros((P, 2, N), dtype=np.float32)
    q2f[tags2, 0, np.arange(N)] = 1.0
    q2f[doclo, 1, np.arange(N)] = 1.0

    qq_full = np.concatenate([q1_full, q2f], axis=1).astype(FP8NP)

    in_maps = []
    for c in range(CORES):
        r = c * ROWS_PER_CORE
        roll = np.r_[r:N, 0:r][:NLOAD]
        qq_c = qq_full[:, :, roll]
        im = {
            "p2": np.ascontiguousarray(
                q2f[:, :, roll[:ROWS_PER_CORE]] * PEN).astype(FP8NP),
        }
        for s, (a, b) in enumerate(STAGES):
            im[f"qs{s}"] = np.ascontiguousarray(qq_c[:, :, a:b])
        in_maps.append(im)

    # Host side: exact partner numerator + the antipodal band (block
    # distance exactly 32), which the device skips.
    z_pair_sim = np.einsum("ij,ij->i", z_i.astype(np.float64),
                           z_j.astype(np.float64))               # [B]
    zb = reps.reshape(N // P, P, D)                              # [64,128,256]
    sim_anti = np.einsum("bij,bkj->bik", zb[:N // P // 2],
                         zb[N // P // 2:]).astype(np.float64)    # [32,128,128]
    e_anti = np.exp(sim_anti / TEMPERATURE)
    tb = tags2.reshape(N // P, P)
    db = docs2.reshape(N // P, P)
    half = N // P // 2
    m = ((tb[:half, :, None] != tb[half:, None, :])
         & (db[:half, :, None] != db[half:, None, :])).astype(np.float64)
    me = m * e_anti
    anti_rowsum = np.concatenate(
        [me.sum(axis=2).reshape(-1), me.sum(axis=1).reshape(-1)])  # [N]
    return in_maps, (z_pair_sim, anti_rowsum)


def _assemble_loss(results, host_extra):
    z_pair_sim, anti_rowsum = host_extra
    rowsum = anti_rowsum.copy()
    w = np.arange(NLOAD)
    for c in range(CORES):
        r = c * ROWS_PER_CORE
        o = np.asarray(results[c]["out"]).astype(np.float64)     # [P, NI]
        rows = r + np.arange(ROWS_PER_CORE)
        rowsum[rows] += o.T.reshape(-1)
        colsum = (np.asarray(results[c]["colE"]).astype(np.float64).sum(0)
                  + np.asarray(results[c]["colO"]).astype(np.float64).sum(0))
        np.add.at(rowsum, (r + w) % N, colsum)
    denom = rowsum + 0.1
    # numerator: exact partner similarity, log(exp(sim/T)) = sim/T
    simfull = np.concatenate([z_pair_sim, z_pair_sim])
    v = np.log(denom) - simfull / TEMPERATURE
    return np.float32(v.sum() / N)


def kernel(emb_i, emb_j, tags, num_classes, document_ids):
    nc = _get_nc()
    in_maps, host_extra = _prepare_inputs(emb_i, emb_j, tags, document_ids)
    res = run_bass_kernel_spmd(nc, in_maps, list(range(CORES)))
    return _assemble_loss(res.results, host_extra)
